# revision 27
# baseline (speedup 1.0000x reference)
"""Trainium2 Bass kernel for nn_LocationSlayerRandom (SLAYER two-branch spiking net).

Contract: kernel(**inputs) takes the FULL unsharded inputs
  spike_input [32,156,1,1,2048] f32, W1 [512,156], W2 [20,512],
  Wl1 [512,2048], Wl2 [20,512], perm [156] i32
and returns the FULL output [32,20,1,1,2204] f32.

Strategy (8 cores, data-parallel over batch, 4 samples/core):

Branch 1 (per sample b): u1 = W1 @ psp_t(si) (psp commutes with the dense).
  - Input psp: DVE tensor_tensor_scan over the fp8 spike rows. Channels
    0:127 scan per-sample; channels 128:155 of all 4 samples pack into one
    [128,T] tile at partition offsets 32b. Scan outputs land fp8 in a
    combo tile laid out [A0|B|A1|B|A2|B|A3|B] (B replicated by SBUF-SBUF
    DMA) so each sample's fc1 runs as a single fp8 DoubleRow pass:
    DR slot 0 = channels 0:127, slot 1 = the packed tail tile against
    per-sample zero-masked weights.
  - Thresholds s1 = (u1>=10) in {0,1} fp8, straight from PSUM: ACT as a
    sharp Sigmoid(50(u1-10)) for most (b,m,half) units, DVE is_ge for the
    last few (engine balancing). Uniform {0,1} encoding means W2 needs no
    scaling and layer-2 thresholds are a plain constant 10.
  - fc2 col-tiled over samples (tile_position=(0,32b)) into one packed
    PSUM, in t-quarters: psp scan from PSUM (DVE) -> o1 = (v>=10) bf16 ->
    output DMA per quarter, so the tail pipelines.

Branch 2: ul1 = psp_c'(Wl1 @ x_tp), x_tp host-gathered+transposed (sipT).
  fc1 on PE in fp8 DoubleRow over the t-contraction into two 1-bank PSUM
  tiles, copied to SBUF by ACT right away (recycles the 2-buf psum2 pool
  without waiting on the busy DVE); the c'-psp runs later as a free-dim
  DVE scan with a reset-pattern data0; l1 = (ul1>=10) bf16; fc2
  col-tiled over samples so the layer-2 psp scans 156 (not 624) elements.

Numerics: all matmuls fp8 with fp32 accumulate (weights and psp inputs
rounded to e4m3). The only nonlinearity is the >=10 threshold; potentials
sit ~6 sigma below it, and layer-2 margins are >6, so the fp8 rounding
(<=0.5 absolute on psp, ~0.1 rms on u1) cannot flip any output bit.
Output DRAM is bf16 ({0,1} exact); host casts to f32.
"""

from contextlib import ExitStack

import numpy as np
import ml_dtypes

import concourse.bass as bass
import concourse.mybir as mybir
from concourse import bacc
from concourse import tile as tile_mod
from concourse.bass_utils import run_bass_kernel_spmd

F32 = mybir.dt.float32
BF16 = mybir.dt.bfloat16
FP8 = mybir.dt.float8e4
AL = mybir.AluOpType
AF = mybir.ActivationFunctionType
BF16_NP = ml_dtypes.bfloat16
FP8_NP = ml_dtypes.float8_e4m3
DR = mybir.MatmulPerfMode.DoubleRow

B, C_IN, T = 32, 156, 2048
HID, OUT_DIM = 512, 20
CP = 156                      # permuted taxel axis (branch-2 "time")
N_CORES = 8
B_PER = B // N_CORES          # 4 samples per core
ALPHA = float(np.exp(-1.0 / 10.0))
THETA = 10.0
NB2 = B_PER * CP              # 624, branch-2 packed free dim
KT = T // 128                 # 16 k-tiles over t
SIG_SCALE = 50.0              # sharp sigmoid ~ exact step at margin >=0.3


def build_program(tc, outs, ins):
    nc = tc.nc
    out = outs["out"]

    with ExitStack() as ctx:
        consts = ctx.enter_context(tc.tile_pool(name="consts", bufs=1))
        work = ctx.enter_context(tc.tile_pool(name="work", bufs=1))
        mid = ctx.enter_context(tc.tile_pool(name="mid", bufs=4))
        psum1 = ctx.enter_context(tc.tile_pool(name="psum1", bufs=3, space="PSUM"))
        psum2 = ctx.enter_context(tc.tile_pool(name="psum2", bufs=1, space="PSUM"))

        # ---------------- constants (gpsimd memsets) ---------------------
        # order matters: junk first (PE warmup gate), then the first alpha
        # half (first-scan gate), then the rest.
        junk = consts.tile([128, 512], FP8, tag="junk")
        nc.gpsimd.memset(junk[:], 1.0)
        bias_sig = consts.tile([128, 1], F32, tag="bsig")
        nc.gpsimd.memset(bias_sig[:], -SIG_SCALE * THETA)
        alpha_t = consts.tile([128, T], F32, tag="alpha")
        nc.gpsimd.memset(alpha_t[:, 0:1024], ALPHA)
        nc.gpsimd.memset(alpha_t[:, 1024:T], ALPHA)
        pat624 = consts.tile([128, NB2], F32, tag="pat624")
        nc.gpsimd.memset(pat624[:], ALPHA)
        for j in range(B_PER):
            nc.gpsimd.memset(pat624[:, j * CP:j * CP + 1], 0.0)
        # load the sigmoid table early (ACT_TABLE_LOAD happens here)
        act_warm = consts.tile([128, 1], F32, tag="actwarm")
        nc.scalar.activation(act_warm[:], bias_sig[:], AF.Sigmoid,
                             bias=bias_sig[:], scale=SIG_SCALE)

        # ---------------- input DMAs (order = trigger order) -------------
        siA = consts.tile([128, B_PER * T], FP8, tag="siA")
        siB = consts.tile([128, T], FP8, tag="siB")
        nc.sync.dma_start(siA[:, 0:1024], ins["siA"][:, 0:1024])
        nc.sync.dma_start(siB[:, 0:1024], ins["siB"][:, 0:1024])
        nc.sync.dma_start(siA[:, 1024:T], ins["siA"][:, 1024:T])
        nc.sync.dma_start(siB[:, 1024:T], ins["siB"][:, 1024:T])
        w1dr = consts.tile([128, 16 * 256], FP8, tag="w1dr")
        nc.sync.dma_start(w1dr[:], ins["W1dr"][:])
        nc.sync.dma_start(siA[:, T:2 * T], ins["siA"][:, T:2 * T])
        nc.sync.dma_start(siA[:, 2 * T:3 * T], ins["siA"][:, 2 * T:3 * T])
        sip = consts.tile([128, KT * NB2], FP8, tag="sip")
        nc.sync.dma_start(sip[:], ins["sipT"][:])
        wl1 = consts.tile([128, KT * HID], FP8, tag="wl1")
        nc.sync.dma_start(wl1[:], ins["Wl1T"][:])
        nc.sync.dma_start(siA[:, 3 * T:4 * T], ins["siA"][:, 3 * T:4 * T])
        w2t = consts.tile([128, 4 * 32], FP8, tag="w2t")
        nc.sync.dma_start(w2t[:], ins["W2T"][:])
        wl2 = consts.tile([128, 4 * 32], BF16, tag="wl2")
        nc.sync.dma_start(wl2[:], ins["Wl2T"][:])

        # ---------------- PE warmup (HAM cold-start) ----------------------
        # ~32 junk matmuls keep the PE busy from ~0.5us so the HAM window
        # flips to 8/8 before real work and never re-throttles.
        pwarm = psum2.tile([128, 512], F32, tag="ps2", name="pwarm")
        for i in range(28):
            nc.tensor.matmul(pwarm[:], junk[:, 0:128], junk[:],
                             start=True, stop=True)

        # ---------------- branch-1 input psp scans (DVE, fp8 out) --------
        # combo slots: [A0 | B | A1 | B | A2 | B | A3 | B] so sample b's
        # fc1 DR rhs is combo[:, 2bT:2bT+2T] viewed as [128, 2, T].
        # A0/B run as interleaved halves (fp8 carry) so fc1(b0)'s h0 units
        # unblock ~5us earlier.
        combo = work.tile([128, 8 * T], FP8, tag="combo")
        H = 1024

        def emit_scan_half(b, hf):
            # slot 2b = A_b; scan half hf with carried initial
            c0 = 2 * b * T + hf * H
            init = 0.0 if hf == 0 else combo[:, c0 - 1:c0]
            nc.vector.tensor_tensor_scan(combo[:, c0:c0 + H], alpha_t[:, 0:H],
                                         siA[:, b * T + hf * H:b * T + (hf + 1) * H],
                                         init, AL.mult, AL.add)

        def emit_scan_B(hf):
            c0 = T + hf * H
            init = 0.0 if hf == 0 else combo[:, c0 - 1:c0]
            nc.vector.tensor_tensor_scan(combo[:, c0:c0 + H], alpha_t[:, 0:H],
                                         siB[:, hf * H:(hf + 1) * H],
                                         init, AL.mult, AL.add)

        # A0/B as interleaved halves (fc1-b0 h0 units unblock early),
        # then A1..A3; B-slot replication after B completes.
        emit_scan_half(0, 0)
        emit_scan_B(0)
        emit_scan_half(0, 1)
        emit_scan_B(1)
        for b in range(1, B_PER):
            emit_scan_half(b, 0)
            emit_scan_half(b, 1)
        for b in range(1, B_PER):
            nc.sync.dma_start(combo[:, (2 * b + 1) * T:(2 * b + 2) * T],
                              combo[:, T:2 * T])

        w1dr4 = w1dr[:].rearrange("p (g j i) -> p g j i", g=16, j=2, i=128)
        combo3 = combo[:].rearrange("p (s t) -> p s t", t=T)
        wl1_3d = wl1[:].rearrange("p (k o) -> p k o", o=HID)
        sip_3d = sip[:].rearrange("p (k c) -> p k c", c=NB2)

        sg = [work.tile([128, 4 * T], FP8, tag=f"sg{b}", name=f"sg{b}")
              for b in range(B_PER)]
        l1 = []

        # ---------------- block emitters ---------------------------------
        # threshold units handed to DVE (late per-sample units; DVE frees up
        # once the input scans drain, ACT is otherwise the pace-setter)
        DVE_UNITS = {(2, 3, 0), (2, 3, 1), (3, 1, 1), (3, 2, 1), (3, 3, 1)}

        def fc1_half(b, hf):
            """fc1 sample b, half hf: 4 m-tiles, fp8 DR, threshold."""
            if True:
                for m in range(4):
                    pu = psum1.tile([128, 1024], F32, tag="ps",
                                    name=f"fc1_{b}{m}{hf}")
                    for ch in range(2):
                        csl = slice(hf * 1024 + ch * 512,
                                    hf * 1024 + (ch + 1) * 512)
                        nc.tensor.matmul(pu[:, ch * 512:(ch + 1) * 512],
                                         w1dr4[:, b * 4 + m],
                                         combo3[:, 2 * b:2 * b + 2, csl],
                                         start=True, stop=True, perf_mode=DR)
                    dst = sg[b][:, m * T + hf * 1024:m * T + (hf + 1) * 1024]
                    if (b, m, hf) in DVE_UNITS:
                        nc.vector.tensor_scalar(dst, pu[:], THETA, None,
                                                AL.is_ge)
                    else:
                        nc.scalar.activation(dst, pu[:], AF.Sigmoid,
                                             bias=bias_sig[:], scale=SIG_SCALE)

        a1sb = [None] * 4
        a1ps = []

        def a1_block(m, copy_eng="act"):
            """branch-2 fc1 m-tile: fp8 DR over t into two 1-bank PSUM
            tiles; copied to SBUF immediately (recycling psum2) by ACT for
            the early blocks, by DVE (its post-scan idle window) for the
            late ones; the psp scan + l1 threshold run later (emit_ul1)."""
            pa = psum2.tile([128, 512], F32, tag="ps2", name=f"pa{m}a")
            pb = psum2.tile([128, 512], F32, tag="ps2", name=f"pa{m}b")
            msl = slice(m * 128, (m + 1) * 128)
            for ki in range(KT // 2):
                st, sp = (ki == 0), (ki == KT // 2 - 1)
                lhs = wl1_3d[:, 2 * ki:2 * ki + 2, msl]
                nc.tensor.matmul(pa[:], lhs,
                                 sip_3d[:, 2 * ki:2 * ki + 2, 0:512],
                                 start=st, stop=sp, perf_mode=DR)
                nc.tensor.matmul(pb[:, 0:NB2 - 512], lhs,
                                 sip_3d[:, 2 * ki:2 * ki + 2, 512:NB2],
                                 start=st, stop=sp, perf_mode=DR)
            a1ps.append((pa, pb))
            if copy_eng is not None:
                emit_a1_copy(m, copy_eng)

        def emit_a1_copy(m, eng):
            pa, pb = a1ps[m]
            ab = mid.tile([128, NB2], F32, tag="a1c", name=f"a1c{m}")
            if eng == "act":
                nc.scalar.activation(ab[:, 0:512], pa[:], AF.Copy)
                nc.scalar.activation(ab[:, 512:NB2], pb[:, 0:NB2 - 512],
                                     AF.Copy)
            else:
                nc.vector.tensor_copy(ab[:, 0:512], pa[:])
                nc.vector.tensor_copy(ab[:, 512:NB2], pb[:, 0:NB2 - 512])
            a1sb[m] = ab

        def emit_ul1(m):
            u = mid.tile([128, NB2], F32, tag="ul1", name=f"ul1{m}")
            nc.vector.tensor_tensor_scan(u[:], pat624[:], a1sb[m][:], 0.0,
                                         AL.mult, AL.add)
            lt = mid.tile([128, NB2], BF16, tag="l1", name=f"l1{m}")
            nc.vector.tensor_scalar(lt[:], u[:], THETA, None, AL.is_ge)
            l1.append(lt)

        # ---------------- main schedule -----------------------------------
        # DVE queue: A0/B halves, A1, ul1-0, A2, ul1-1, A3, ul1-2, (b2/b3
        # DVE threshold units via fc1_block), ul1-3 -- ul1 scans slot into
        # the scan chain right after their a1 block lands, which also
        # recycles the two psum2 bufs for the next a1 block.
        # per-sample h0+h1 units with a1 blocks filling the PE idle left
        # by the consumer-paced fc1 pipeline.
        fc1_half(0, 0)
        fc1_half(0, 1)
        a1_block(0)
        a1_block(1)
        fc1_half(1, 0)
        fc1_half(1, 1)
        a1_block(2)
        emit_ul1(0)
        emit_ul1(1)
        fc1_half(2, 0)
        fc1_half(2, 1)
        a1_block(3)
        emit_ul1(2)
        fc1_half(3, 0)
        fc1_half(3, 1)
        emit_ul1(3)
        # ---------------- branch-1 fc2 + psp + out, in t-quarters ---------
        vs = work.tile([128, T], F32, tag="vs")
        o1 = work.tile([128, T], BF16, tag="o1")

        def emit_quarter(q):
            qsl = slice(q * 512, (q + 1) * 512)
            puq = psum1.tile([128, 1024], F32, tag="ps", name=f"fc2q{q}")
            for m in range(4):
                for b in range(B_PER):
                    nc.tensor.matmul(puq[32 * b:32 * b + 32, 0:512],
                                     w2t[:, m * 32:(m + 1) * 32],
                                     sg[b][:, m * T + q * 512:m * T + (q + 1) * 512],
                                     start=(m == 0), stop=(m == 3),
                                     tile_position=(0, 32 * b),
                                     skip_group_check=True)
            init = 0.0 if q == 0 else vs[:, q * 512 - 1:q * 512]
            nc.vector.tensor_tensor_scan(vs[:, qsl], alpha_t[:, 0:512],
                                         puq[:, 0:512], init,
                                         AL.mult, AL.add)
            if q == 3:
                nc.vector.tensor_scalar(o1[:, qsl], vs[:, qsl], THETA, None,
                                        AL.is_ge)
            else:
                nc.scalar.activation(o1[:, qsl], vs[:, qsl], AF.Sigmoid,
                                     bias=bias_sig[:], scale=SIG_SCALE)
            nc.sync.dma_start(
                out[:, :, qsl].rearrange("b j t -> (b j) t"), o1[:, qsl])

        # ---------------- branch-2 fc2 (col-tiled) + psp + out ------------
        pl2 = psum2.tile([128, 512], F32, tag="ps2", name="pl2")
        for k in range(4):
            for b in range(B_PER):
                nc.tensor.matmul(pl2[32 * b:32 * b + 32, 0:CP],
                                 wl2[:, k * 32:(k + 1) * 32],
                                 l1[k][:, b * CP:(b + 1) * CP],
                                 start=(k == 0), stop=(k == 3),
                                 tile_position=(0, 32 * b),
                                 skip_group_check=True)
        u2 = mid.tile([128, CP], F32, tag="u2")
        nc.vector.tensor_tensor_scan(u2[:], alpha_t[:, 0:CP], pl2[:, 0:CP],
                                     0.0, AL.mult, AL.add)
        o2 = mid.tile([128, CP], BF16, tag="o2")
        nc.scalar.activation(o2[:], u2[:], AF.Sigmoid,
                             bias=bias_sig[:], scale=SIG_SCALE)
        nc.sync.dma_start(
            out[:, :, T:T + CP].rearrange("b j c -> (b j) c"), o2[:])

        for q in range(4):
            emit_quarter(q)


# ======================= host-side preparation =======================

def prep_core_inputs(si, sip, core):
    """Per-core data tensors (fp8), pre-packed into single-DMA layouts."""
    sl = si[core * B_PER:(core + 1) * B_PER]          # [4,156,2048]
    siA = np.ascontiguousarray(
        sl[:, :128, :].transpose(1, 0, 2).reshape(128, B_PER * T)
    ).astype(FP8_NP)
    siB = np.zeros((128, T), dtype=FP8_NP)
    for b in range(B_PER):
        siB[32 * b:32 * b + (C_IN - 128)] = sl[b, 128:C_IN, :]
    sp = sip[core * B_PER:(core + 1) * B_PER]         # [4,156,2048]
    sipT = np.ascontiguousarray(
        sp.transpose(2, 0, 1).reshape(KT, 128, NB2)
        .transpose(1, 0, 2).reshape(128, KT * NB2)
    ).astype(FP8_NP)
    return {"siA": siA, "siB": siB, "sipT": sipT}


def prep_shared_inputs(W1, W2, Wl1, Wl2):
    """Weight layouts, shared by all cores (all fp8 except Wl2 bf16)."""
    w1t = W1.T.astype(np.float32)                     # [156, 512]
    W1dr = np.zeros((128, 4, 4, 2, 128), dtype=np.float32)
    for b in range(B_PER):
        for m in range(4):
            W1dr[:, b, m, 0, :] = w1t[:128, m * 128:(m + 1) * 128]
            W1dr[32 * b:32 * b + (C_IN - 128), b, m, 1, :] = \
                w1t[128:C_IN, m * 128:(m + 1) * 128]
    W1dr = W1dr.reshape(128, 16 * 256).astype(FP8_NP)

    w2tt = W2.T.astype(np.float32)                    # [512, 20]
    W2T = np.zeros((128, 4 * 32), dtype=FP8_NP)
    for k in range(4):
        W2T[:, k * 32:k * 32 + OUT_DIM] = w2tt[k * 128:(k + 1) * 128]

    Wl1T = np.ascontiguousarray(
        Wl1.T.reshape(KT, 128, HID).transpose(1, 0, 2).reshape(128, KT * HID)
    ).astype(FP8_NP)

    wl2t = Wl2.T.astype(np.float32)                   # [512, 20]
    Wl2T = np.zeros((128, 4 * 32), dtype=BF16_NP)
    for k in range(4):
        Wl2T[:, k * 32:k * 32 + OUT_DIM] = wl2t[k * 128:(k + 1) * 128]
    return {"W1dr": W1dr, "W2T": W2T, "Wl1T": Wl1T, "Wl2T": Wl2T}


def make_in_maps(spike_input, W1, W2, Wl1, Wl2, perm):
    si = np.asarray(spike_input, dtype=np.float32).reshape(B, C_IN, T)
    perm = np.asarray(perm).astype(np.int64)
    sip = si[:, perm, :]                              # perm-gather (layout only)
    shared = prep_shared_inputs(np.asarray(W1, np.float32),
                                np.asarray(W2, np.float32),
                                np.asarray(Wl1, np.float32),
                                np.asarray(Wl2, np.float32))
    in_maps = []
    for core in range(N_CORES):
        m = dict(shared)
        m.update(prep_core_inputs(si, sip, core))
        in_maps.append(m)
    return in_maps


_IN_SPECS = {
    "siA": ((128, B_PER * T), FP8),
    "siB": ((128, T), FP8),
    "sipT": ((128, KT * NB2), FP8),
    "W1dr": ((128, 16 * 256), FP8),
    "W2T": ((128, 4 * 32), FP8),
    "Wl1T": ((128, KT * HID), FP8),
    "Wl2T": ((128, 4 * 32), BF16),
}


def build_bass():
    nc = bacc.Bacc("TRN2", target_bir_lowering=False, debug=False)
    ins = {}
    for name, (shape, dt) in _IN_SPECS.items():
        h = nc.dram_tensor(name, list(shape), dt, kind="ExternalInput")
        ins[name] = h[:]
    out_h = nc.dram_tensor("out", [B_PER, 32, T + CP], BF16,
                           kind="ExternalOutput")
    outs = {"out": out_h[:]}
    with tile_mod.TileContext(nc) as tc:
        build_program(tc, outs, ins)
    nc.compile()
    return nc


_NC_CACHE = None


def run(inputs, trace=False, **kw):
    """Run on the 8 NeuronCores; returns (full_output, BassKernelResults)."""
    global _NC_CACHE
    if _NC_CACHE is None:
        _NC_CACHE = build_bass()
    nc = _NC_CACHE
    in_maps = make_in_maps(**inputs)
    res = run_bass_kernel_spmd(nc, in_maps, core_ids=list(range(N_CORES)),
                               trace=trace, **kw)
    parts = [res.results[c]["out"][:, :OUT_DIM, :] for c in range(N_CORES)]
    full = np.concatenate(parts, axis=0).reshape(B, OUT_DIM, 1, 1, T + CP)
    return np.ascontiguousarray(full.astype(np.float32)), res


def kernel(**inputs):
    out, _ = run(inputs)
    return out


# revision 28
# speedup vs baseline: 1.0859x; 1.0859x over previous
"""Trainium2 Bass kernel for nn_LocationSlayerRandom (SLAYER two-branch spiking net).

Contract: kernel(**inputs) takes the FULL unsharded inputs
  spike_input [32,156,1,1,2048] f32, W1 [512,156], W2 [20,512],
  Wl1 [512,2048], Wl2 [20,512], perm [156] i32
and returns the FULL output [32,20,1,1,2204] f32.

Strategy (8 cores, data-parallel over batch, 4 samples/core):

Branch 1 (per sample b): u1 = W1 @ psp_t(si) (psp commutes with the dense).
  - Input psp: DVE tensor_tensor_scan over the fp8 spike rows. Channels
    0:127 scan per-sample; channels 128:155 of all 4 samples pack into one
    [128,T] tile at partition offsets 32b. Scan outputs land fp8 in a
    combo tile laid out [A0|B|A1|B|A2|B|A3|B] (B replicated by SBUF-SBUF
    DMA) so each sample's fc1 runs as a single fp8 DoubleRow pass:
    DR slot 0 = channels 0:127, slot 1 = the packed tail tile against
    per-sample zero-masked weights.
  - Thresholds s1 = (u1>=10) in {0,1} fp8, straight from PSUM: ACT as a
    sharp Sigmoid(50(u1-10)) for most (b,m,half) units, DVE is_ge for the
    last few (engine balancing). Uniform {0,1} encoding means W2 needs no
    scaling and layer-2 thresholds are a plain constant 10.
  - fc2 col-tiled over samples (tile_position=(0,32b)) into one packed
    PSUM, in t-quarters: psp scan from PSUM (DVE) -> o1 = (v>=10) bf16 ->
    output DMA per quarter, so the tail pipelines.

Branch 2: ul1 = psp_c'(Wl1 @ x_tp), x_tp host-gathered+transposed (sipT).
  fc1 on PE in fp8 DoubleRow over the t-contraction into two 1-bank PSUM
  tiles, copied to SBUF by ACT right away (recycles the 2-buf psum2 pool
  without waiting on the busy DVE); the c'-psp runs later as a free-dim
  DVE scan with a reset-pattern data0; l1 = (ul1>=10) bf16; fc2
  col-tiled over samples so the layer-2 psp scans 156 (not 624) elements.

Numerics: all matmuls fp8 with fp32 accumulate (weights and psp inputs
rounded to e4m3). The only nonlinearity is the >=10 threshold; potentials
sit ~6 sigma below it, and layer-2 margins are >6, so the fp8 rounding
(<=0.5 absolute on psp, ~0.1 rms on u1) cannot flip any output bit.
Output DRAM is bf16 ({0,1} exact); host casts to f32.
"""

from contextlib import ExitStack

import numpy as np
import ml_dtypes

import concourse.bass as bass
import concourse.mybir as mybir
from concourse import bacc
from concourse import tile as tile_mod
from concourse.bass_utils import run_bass_kernel_spmd

F32 = mybir.dt.float32
BF16 = mybir.dt.bfloat16
FP8 = mybir.dt.float8e4
AL = mybir.AluOpType
AF = mybir.ActivationFunctionType
BF16_NP = ml_dtypes.bfloat16
FP8_NP = ml_dtypes.float8_e4m3
DR = mybir.MatmulPerfMode.DoubleRow

B, C_IN, T = 32, 156, 2048
HID, OUT_DIM = 512, 20
CP = 156                      # permuted taxel axis (branch-2 "time")
N_CORES = 8
B_PER = B // N_CORES          # 4 samples per core
ALPHA = float(np.exp(-1.0 / 10.0))
THETA = 10.0
NB2 = B_PER * CP              # 624, branch-2 packed free dim
KT = T // 128                 # 16 k-tiles over t
SIG_SCALE = 50.0              # sharp sigmoid ~ exact step at margin >=0.3


def build_program(tc, outs, ins):
    nc = tc.nc
    out = outs["out"]

    with ExitStack() as ctx:
        consts = ctx.enter_context(tc.tile_pool(name="consts", bufs=1))
        work = ctx.enter_context(tc.tile_pool(name="work", bufs=1))
        mid = ctx.enter_context(tc.tile_pool(name="mid", bufs=4))
        psum1 = ctx.enter_context(tc.tile_pool(name="psum1", bufs=3, space="PSUM"))
        psum2 = ctx.enter_context(tc.tile_pool(name="psum2", bufs=1, space="PSUM"))

        # ---------------- constants (gpsimd memsets) ---------------------
        # order matters: junk first (PE warmup gate), then the first alpha
        # half (first-scan gate), then the rest.
        junk = consts.tile([128, 512], FP8, tag="junk")
        nc.gpsimd.memset(junk[:], 1.0)
        bias_sig = consts.tile([128, 1], F32, tag="bsig")
        nc.gpsimd.memset(bias_sig[:], -SIG_SCALE * THETA)
        alpha_t = consts.tile([128, T], F32, tag="alpha")
        nc.gpsimd.memset(alpha_t[:, 0:1024], ALPHA)
        nc.gpsimd.memset(alpha_t[:, 1024:T], ALPHA)
        pat624 = consts.tile([128, NB2], F32, tag="pat624")
        nc.gpsimd.memset(pat624[:], ALPHA)
        for j in range(B_PER):
            nc.gpsimd.memset(pat624[:, j * CP:j * CP + 1], 0.0)
        # load the sigmoid table early (ACT_TABLE_LOAD happens here)
        act_warm = consts.tile([128, 1], F32, tag="actwarm")
        nc.scalar.activation(act_warm[:], bias_sig[:], AF.Sigmoid,
                             bias=bias_sig[:], scale=SIG_SCALE)

        # ---------------- input DMAs (order = trigger order) -------------
        siA = consts.tile([128, B_PER * T], FP8, tag="siA")
        siB = consts.tile([128, T], FP8, tag="siB")
        nc.sync.dma_start(siA[:, 0:512], ins["siA"][:, 0:512])
        nc.sync.dma_start(siA[:, 512:1024], ins["siA"][:, 512:1024])
        nc.sync.dma_start(siB[:, 0:1024], ins["siB"][:, 0:1024])
        nc.sync.dma_start(siA[:, 1024:T], ins["siA"][:, 1024:T])
        nc.sync.dma_start(siB[:, 1024:T], ins["siB"][:, 1024:T])
        w1dr = consts.tile([128, 16 * 256], FP8, tag="w1dr")
        nc.sync.dma_start(w1dr[:], ins["W1dr"][:])
        nc.sync.dma_start(siA[:, T:2 * T], ins["siA"][:, T:2 * T])
        nc.sync.dma_start(siA[:, 2 * T:3 * T], ins["siA"][:, 2 * T:3 * T])
        sip = consts.tile([128, KT * NB2], FP8, tag="sip")
        nc.sync.dma_start(sip[:], ins["sipT"][:])
        wl1 = consts.tile([128, KT * HID], FP8, tag="wl1")
        nc.sync.dma_start(wl1[:], ins["Wl1T"][:])
        nc.sync.dma_start(siA[:, 3 * T:4 * T], ins["siA"][:, 3 * T:4 * T])
        w2t = consts.tile([128, 4 * 32], FP8, tag="w2t")
        nc.sync.dma_start(w2t[:], ins["W2T"][:])
        wl2 = consts.tile([128, 4 * 32], BF16, tag="wl2")
        nc.sync.dma_start(wl2[:], ins["Wl2T"][:])

        # ---------------- PE warmup (HAM cold-start) ----------------------
        # ~32 junk matmuls keep the PE busy from ~0.5us so the HAM window
        # flips to 8/8 before real work and never re-throttles.
        pwarm = psum2.tile([128, 512], F32, tag="ps2", name="pwarm")
        for i in range(28):
            nc.tensor.matmul(pwarm[:], junk[:, 0:128], junk[:],
                             start=True, stop=True)

        # ---------------- branch-1 input psp scans (DVE, fp8 out) --------
        # combo slots: [A0 | B | A1 | B | A2 | B | A3 | B] so sample b's
        # fc1 DR rhs is combo[:, 2bT:2bT+2T] viewed as [128, 2, T].
        # A0/B run as interleaved halves (fp8 carry) so fc1(b0)'s h0 units
        # unblock ~5us earlier.
        combo = work.tile([128, 8 * T], FP8, tag="combo")
        H = 1024

        def emit_scan_half(b, hf):
            # slot 2b = A_b; scan half hf with carried initial
            c0 = 2 * b * T + hf * H
            init = 0.0 if hf == 0 else combo[:, c0 - 1:c0]
            nc.vector.tensor_tensor_scan(combo[:, c0:c0 + H], alpha_t[:, 0:H],
                                         siA[:, b * T + hf * H:b * T + (hf + 1) * H],
                                         init, AL.mult, AL.add)

        def emit_scan_B(hf):
            c0 = T + hf * H
            init = 0.0 if hf == 0 else combo[:, c0 - 1:c0]
            nc.vector.tensor_tensor_scan(combo[:, c0:c0 + H], alpha_t[:, 0:H],
                                         siB[:, hf * H:(hf + 1) * H],
                                         init, AL.mult, AL.add)

        # A0/B as interleaved halves (fc1-b0 h0 units unblock early),
        # then A1..A3; B-slot replication after B completes. A0's first
        # half runs as two 512 chunks so the chain starts on a smaller DMA.
        nc.vector.tensor_tensor_scan(combo[:, 0:512], alpha_t[:, 0:512],
                                     siA[:, 0:512], 0.0, AL.mult, AL.add)
        nc.vector.tensor_tensor_scan(combo[:, 512:1024], alpha_t[:, 0:512],
                                     siA[:, 512:1024], combo[:, 511:512],
                                     AL.mult, AL.add)
        emit_scan_B(0)
        emit_scan_half(0, 1)
        emit_scan_B(1)
        for b in range(1, B_PER):
            emit_scan_half(b, 0)
            emit_scan_half(b, 1)
        for b in range(1, B_PER):
            nc.sync.dma_start(combo[:, (2 * b + 1) * T:(2 * b + 2) * T],
                              combo[:, T:2 * T])

        w1dr4 = w1dr[:].rearrange("p (g j i) -> p g j i", g=16, j=2, i=128)
        combo3 = combo[:].rearrange("p (s t) -> p s t", t=T)
        wl1_3d = wl1[:].rearrange("p (k o) -> p k o", o=HID)
        sip_3d = sip[:].rearrange("p (k c) -> p k c", c=NB2)

        sg = [work.tile([128, 4 * T], FP8, tag=f"sg{b}", name=f"sg{b}")
              for b in range(B_PER)]
        l1 = []

        # ---------------- block emitters ---------------------------------
        # threshold units handed to DVE (late per-sample units; DVE frees up
        # once the input scans drain, ACT is otherwise the pace-setter)
        DVE_UNITS = {(2, 3, 0), (2, 3, 1), (3, 2, 1), (3, 3, 1)}

        def fc1_half(b, hf):
            """fc1 sample b, half hf: 4 m-tiles, fp8 DR, threshold."""
            if True:
                for m in range(4):
                    pu = psum1.tile([128, 1024], F32, tag="ps",
                                    name=f"fc1_{b}{m}{hf}")
                    for ch in range(2):
                        csl = slice(hf * 1024 + ch * 512,
                                    hf * 1024 + (ch + 1) * 512)
                        nc.tensor.matmul(pu[:, ch * 512:(ch + 1) * 512],
                                         w1dr4[:, b * 4 + m],
                                         combo3[:, 2 * b:2 * b + 2, csl],
                                         start=True, stop=True, perf_mode=DR)
                    dst = sg[b][:, m * T + hf * 1024:m * T + (hf + 1) * 1024]
                    if (b, m, hf) in DVE_UNITS:
                        nc.vector.tensor_scalar(dst, pu[:], THETA, None,
                                                AL.is_ge)
                    else:
                        nc.scalar.activation(dst, pu[:], AF.Sigmoid,
                                             bias=bias_sig[:], scale=SIG_SCALE)

        a1sb = [None] * 4
        a1ps = []

        def a1_block(m, copy_eng="act"):
            """branch-2 fc1 m-tile: fp8 DR over t into two 1-bank PSUM
            tiles; copied to SBUF immediately (recycling psum2) by ACT for
            the early blocks, by DVE (its post-scan idle window) for the
            late ones; the psp scan + l1 threshold run later (emit_ul1)."""
            pa = psum2.tile([128, 512], F32, tag="ps2", name=f"pa{m}a")
            pb = psum2.tile([128, 512], F32, tag="ps2", name=f"pa{m}b")
            msl = slice(m * 128, (m + 1) * 128)
            for ki in range(KT // 2):
                st, sp = (ki == 0), (ki == KT // 2 - 1)
                lhs = wl1_3d[:, 2 * ki:2 * ki + 2, msl]
                nc.tensor.matmul(pa[:], lhs,
                                 sip_3d[:, 2 * ki:2 * ki + 2, 0:512],
                                 start=st, stop=sp, perf_mode=DR)
                nc.tensor.matmul(pb[:, 0:NB2 - 512], lhs,
                                 sip_3d[:, 2 * ki:2 * ki + 2, 512:NB2],
                                 start=st, stop=sp, perf_mode=DR)
            a1ps.append((pa, pb))
            if copy_eng is not None:
                emit_a1_copy(m, copy_eng)

        def emit_a1_copy(m, eng):
            pa, pb = a1ps[m]
            ab = mid.tile([128, NB2], F32, tag="a1c", name=f"a1c{m}")
            if eng == "act":
                nc.scalar.activation(ab[:, 0:512], pa[:], AF.Copy)
                nc.scalar.activation(ab[:, 512:NB2], pb[:, 0:NB2 - 512],
                                     AF.Copy)
            else:
                nc.vector.tensor_copy(ab[:, 0:512], pa[:])
                nc.vector.tensor_copy(ab[:, 512:NB2], pb[:, 0:NB2 - 512])
            a1sb[m] = ab

        def emit_ul1(m):
            u = mid.tile([128, NB2], F32, tag="ul1", name=f"ul1{m}")
            nc.vector.tensor_tensor_scan(u[:], pat624[:], a1sb[m][:], 0.0,
                                         AL.mult, AL.add)
            lt = mid.tile([128, NB2], BF16, tag="l1", name=f"l1{m}")
            nc.vector.tensor_scalar(lt[:], u[:], THETA, None, AL.is_ge)
            l1.append(lt)

        # ---------------- main schedule -----------------------------------
        # DVE queue: A0/B halves, A1, ul1-0, A2, ul1-1, A3, ul1-2, (b2/b3
        # DVE threshold units via fc1_block), ul1-3 -- ul1 scans slot into
        # the scan chain right after their a1 block lands, which also
        # recycles the two psum2 bufs for the next a1 block.
        # per-sample h0+h1 units with a1 blocks filling the PE idle left
        # by the consumer-paced fc1 pipeline.
        fc1_half(0, 0)
        fc1_half(0, 1)
        a1_block(0)
        a1_block(1)
        fc1_half(1, 0)
        fc1_half(1, 1)
        a1_block(2)
        emit_ul1(0)
        emit_ul1(1)
        fc1_half(2, 0)
        fc1_half(2, 1)
        a1_block(3)
        emit_ul1(2)
        fc1_half(3, 0)
        fc1_half(3, 1)
        emit_ul1(3)
        # ---------------- branch-1 fc2 + psp + out, in t-quarters ---------
        vs = work.tile([128, T], F32, tag="vs")
        o1 = work.tile([128, T], BF16, tag="o1")

        def emit_quarter(q):
            qsl = slice(q * 512, (q + 1) * 512)
            puq = psum1.tile([128, 1024], F32, tag="ps", name=f"fc2q{q}")
            for m in range(4):
                for b in range(B_PER):
                    nc.tensor.matmul(puq[32 * b:32 * b + 32, 0:512],
                                     w2t[:, m * 32:(m + 1) * 32],
                                     sg[b][:, m * T + q * 512:m * T + (q + 1) * 512],
                                     start=(m == 0), stop=(m == 3),
                                     tile_position=(0, 32 * b),
                                     skip_group_check=True)
            init = 0.0 if q == 0 else vs[:, q * 512 - 1:q * 512]
            nc.vector.tensor_tensor_scan(vs[:, qsl], alpha_t[:, 0:512],
                                         puq[:, 0:512], init,
                                         AL.mult, AL.add)
            if q == 3:
                nc.vector.tensor_scalar(o1[:, qsl], vs[:, qsl], THETA, None,
                                        AL.is_ge)
            else:
                nc.scalar.activation(o1[:, qsl], vs[:, qsl], AF.Sigmoid,
                                     bias=bias_sig[:], scale=SIG_SCALE)
            nc.sync.dma_start(
                out[:, :, qsl].rearrange("b j t -> (b j) t"), o1[:, qsl])

        # ---------------- branch-2 fc2 (col-tiled) + psp + out ------------
        pl2 = psum2.tile([128, 512], F32, tag="ps2", name="pl2")
        for k in range(4):
            for b in range(B_PER):
                nc.tensor.matmul(pl2[32 * b:32 * b + 32, 0:CP],
                                 wl2[:, k * 32:(k + 1) * 32],
                                 l1[k][:, b * CP:(b + 1) * CP],
                                 start=(k == 0), stop=(k == 3),
                                 tile_position=(0, 32 * b),
                                 skip_group_check=True)
        u2 = mid.tile([128, CP], F32, tag="u2")
        nc.vector.tensor_tensor_scan(u2[:], alpha_t[:, 0:CP], pl2[:, 0:CP],
                                     0.0, AL.mult, AL.add)
        o2 = mid.tile([128, CP], BF16, tag="o2")
        nc.scalar.activation(o2[:], u2[:], AF.Sigmoid,
                             bias=bias_sig[:], scale=SIG_SCALE)
        nc.sync.dma_start(
            out[:, :, T:T + CP].rearrange("b j c -> (b j) c"), o2[:])

        for q in range(4):
            emit_quarter(q)


# ======================= host-side preparation =======================

def prep_core_inputs(si, sip, core):
    """Per-core data tensors (fp8), pre-packed into single-DMA layouts."""
    sl = si[core * B_PER:(core + 1) * B_PER]          # [4,156,2048]
    siA = np.ascontiguousarray(
        sl[:, :128, :].transpose(1, 0, 2).reshape(128, B_PER * T)
    ).astype(FP8_NP)
    siB = np.zeros((128, T), dtype=FP8_NP)
    for b in range(B_PER):
        siB[32 * b:32 * b + (C_IN - 128)] = sl[b, 128:C_IN, :]
    sp = sip[core * B_PER:(core + 1) * B_PER]         # [4,156,2048]
    sipT = np.ascontiguousarray(
        sp.transpose(2, 0, 1).reshape(KT, 128, NB2)
        .transpose(1, 0, 2).reshape(128, KT * NB2)
    ).astype(FP8_NP)
    return {"siA": siA, "siB": siB, "sipT": sipT}


def prep_shared_inputs(W1, W2, Wl1, Wl2):
    """Weight layouts, shared by all cores (all fp8 except Wl2 bf16)."""
    w1t = W1.T.astype(np.float32)                     # [156, 512]
    W1dr = np.zeros((128, 4, 4, 2, 128), dtype=np.float32)
    for b in range(B_PER):
        for m in range(4):
            W1dr[:, b, m, 0, :] = w1t[:128, m * 128:(m + 1) * 128]
            W1dr[32 * b:32 * b + (C_IN - 128), b, m, 1, :] = \
                w1t[128:C_IN, m * 128:(m + 1) * 128]
    W1dr = W1dr.reshape(128, 16 * 256).astype(FP8_NP)

    w2tt = W2.T.astype(np.float32)                    # [512, 20]
    W2T = np.zeros((128, 4 * 32), dtype=FP8_NP)
    for k in range(4):
        W2T[:, k * 32:k * 32 + OUT_DIM] = w2tt[k * 128:(k + 1) * 128]

    Wl1T = np.ascontiguousarray(
        Wl1.T.reshape(KT, 128, HID).transpose(1, 0, 2).reshape(128, KT * HID)
    ).astype(FP8_NP)

    wl2t = Wl2.T.astype(np.float32)                   # [512, 20]
    Wl2T = np.zeros((128, 4 * 32), dtype=BF16_NP)
    for k in range(4):
        Wl2T[:, k * 32:k * 32 + OUT_DIM] = wl2t[k * 128:(k + 1) * 128]
    return {"W1dr": W1dr, "W2T": W2T, "Wl1T": Wl1T, "Wl2T": Wl2T}


def make_in_maps(spike_input, W1, W2, Wl1, Wl2, perm):
    si = np.asarray(spike_input, dtype=np.float32).reshape(B, C_IN, T)
    perm = np.asarray(perm).astype(np.int64)
    sip = si[:, perm, :]                              # perm-gather (layout only)
    shared = prep_shared_inputs(np.asarray(W1, np.float32),
                                np.asarray(W2, np.float32),
                                np.asarray(Wl1, np.float32),
                                np.asarray(Wl2, np.float32))
    in_maps = []
    for core in range(N_CORES):
        m = dict(shared)
        m.update(prep_core_inputs(si, sip, core))
        in_maps.append(m)
    return in_maps


_IN_SPECS = {
    "siA": ((128, B_PER * T), FP8),
    "siB": ((128, T), FP8),
    "sipT": ((128, KT * NB2), FP8),
    "W1dr": ((128, 16 * 256), FP8),
    "W2T": ((128, 4 * 32), FP8),
    "Wl1T": ((128, KT * HID), FP8),
    "Wl2T": ((128, 4 * 32), BF16),
}


def build_bass():
    nc = bacc.Bacc("TRN2", target_bir_lowering=False, debug=False)
    ins = {}
    for name, (shape, dt) in _IN_SPECS.items():
        h = nc.dram_tensor(name, list(shape), dt, kind="ExternalInput")
        ins[name] = h[:]
    out_h = nc.dram_tensor("out", [B_PER, 32, T + CP], BF16,
                           kind="ExternalOutput")
    outs = {"out": out_h[:]}
    with tile_mod.TileContext(nc) as tc:
        build_program(tc, outs, ins)
    nc.compile()
    return nc


_NC_CACHE = None


def run(inputs, trace=False, **kw):
    """Run on the 8 NeuronCores; returns (full_output, BassKernelResults)."""
    global _NC_CACHE
    if _NC_CACHE is None:
        _NC_CACHE = build_bass()
    nc = _NC_CACHE
    in_maps = make_in_maps(**inputs)
    res = run_bass_kernel_spmd(nc, in_maps, core_ids=list(range(N_CORES)),
                               trace=trace, **kw)
    parts = [res.results[c]["out"][:, :OUT_DIM, :] for c in range(N_CORES)]
    full = np.concatenate(parts, axis=0).reshape(B, OUT_DIM, 1, 1, T + CP)
    return np.ascontiguousarray(full.astype(np.float32)), res


def kernel(**inputs):
    out, _ = run(inputs)
    return out


# revision 29
# speedup vs baseline: 1.1032x; 1.0159x over previous
"""Trainium2 Bass kernel for nn_LocationSlayerRandom (SLAYER two-branch spiking net).

Contract: kernel(**inputs) takes the FULL unsharded inputs
  spike_input [32,156,1,1,2048] f32, W1 [512,156], W2 [20,512],
  Wl1 [512,2048], Wl2 [20,512], perm [156] i32
and returns the FULL output [32,20,1,1,2204] f32.

Strategy (8 cores, data-parallel over batch, 4 samples/core):

Branch 1 (per sample b): u1 = W1 @ psp_t(si) (psp commutes with the dense).
  - Input psp: DVE tensor_tensor_scan over the fp8 spike rows. Channels
    0:127 scan per-sample; channels 128:155 of all 4 samples pack into one
    [128,T] tile at partition offsets 32b. Scan outputs land fp8 in a
    combo tile laid out [A0|B|A1|B|A2|B|A3|B] (B replicated by SBUF-SBUF
    DMA) so each sample's fc1 runs as a single fp8 DoubleRow pass:
    DR slot 0 = channels 0:127, slot 1 = the packed tail tile against
    per-sample zero-masked weights.
  - Thresholds s1 = (u1>=10) in {0,1} fp8, straight from PSUM: ACT as a
    sharp Sigmoid(50(u1-10)) for most (b,m,half) units, DVE is_ge for the
    last few (engine balancing). Uniform {0,1} encoding means W2 needs no
    scaling and layer-2 thresholds are a plain constant 10.
  - fc2 col-tiled over samples (tile_position=(0,32b)) into one packed
    PSUM, in t-quarters: psp scan from PSUM (DVE) -> o1 = (v>=10) bf16 ->
    output DMA per quarter, so the tail pipelines.

Branch 2: ul1 = psp_c'(Wl1 @ x_tp), x_tp host-gathered+transposed (sipT).
  fc1 on PE in fp8 DoubleRow over the t-contraction into two 1-bank PSUM
  tiles, copied to SBUF by ACT right away (recycles the 2-buf psum2 pool
  without waiting on the busy DVE); the c'-psp runs later as a free-dim
  DVE scan with a reset-pattern data0; l1 = (ul1>=10) bf16; fc2
  col-tiled over samples so the layer-2 psp scans 156 (not 624) elements.

Numerics: all matmuls fp8 with fp32 accumulate (weights and psp inputs
rounded to e4m3). The only nonlinearity is the >=10 threshold; potentials
sit ~6 sigma below it, and layer-2 margins are >6, so the fp8 rounding
(<=0.5 absolute on psp, ~0.1 rms on u1) cannot flip any output bit.
Output DRAM is bf16 ({0,1} exact); host casts to f32.
"""

from contextlib import ExitStack

import numpy as np
import ml_dtypes

import concourse.bass as bass
import concourse.mybir as mybir
from concourse import bacc
from concourse import tile as tile_mod
from concourse.bass_utils import run_bass_kernel_spmd

F32 = mybir.dt.float32
BF16 = mybir.dt.bfloat16
FP8 = mybir.dt.float8e4
AL = mybir.AluOpType
AF = mybir.ActivationFunctionType
BF16_NP = ml_dtypes.bfloat16
FP8_NP = ml_dtypes.float8_e4m3
DR = mybir.MatmulPerfMode.DoubleRow

B, C_IN, T = 32, 156, 2048
HID, OUT_DIM = 512, 20
CP = 156                      # permuted taxel axis (branch-2 "time")
N_CORES = 8
B_PER = B // N_CORES          # 4 samples per core
ALPHA = float(np.exp(-1.0 / 10.0))
THETA = 10.0
NB2 = B_PER * CP              # 624, branch-2 packed free dim
KT = T // 128                 # 16 k-tiles over t
SIG_SCALE = 50.0              # sharp sigmoid ~ exact step at margin >=0.3


def build_program(tc, outs, ins):
    nc = tc.nc
    out = outs["out"]

    with ExitStack() as ctx:
        consts = ctx.enter_context(tc.tile_pool(name="consts", bufs=1))
        work = ctx.enter_context(tc.tile_pool(name="work", bufs=1))
        mid = ctx.enter_context(tc.tile_pool(name="mid", bufs=4))
        psum1 = ctx.enter_context(tc.tile_pool(name="psum1", bufs=3, space="PSUM"))
        psum2 = ctx.enter_context(tc.tile_pool(name="psum2", bufs=1, space="PSUM"))

        # ---------------- constants (gpsimd memsets) ---------------------
        # order matters: junk first (PE warmup gate), then the first alpha
        # half (first-scan gate), then the rest.
        junk = consts.tile([128, 512], FP8, tag="junk")
        nc.gpsimd.memset(junk[:], 1.0)
        bias_sig = consts.tile([128, 1], F32, tag="bsig")
        nc.gpsimd.memset(bias_sig[:], -SIG_SCALE * THETA)
        alpha_t = consts.tile([128, T], F32, tag="alpha")
        nc.gpsimd.memset(alpha_t[:, 0:1024], ALPHA)
        nc.gpsimd.memset(alpha_t[:, 1024:T], ALPHA)
        pat624 = consts.tile([128, NB2], F32, tag="pat624")
        nc.gpsimd.memset(pat624[:], ALPHA)
        for j in range(B_PER):
            nc.gpsimd.memset(pat624[:, j * CP:j * CP + 1], 0.0)
        # load the sigmoid table early (ACT_TABLE_LOAD happens here)
        act_warm = consts.tile([128, 1], F32, tag="actwarm")
        nc.scalar.activation(act_warm[:], bias_sig[:], AF.Sigmoid,
                             bias=bias_sig[:], scale=SIG_SCALE)

        # ---------------- input DMAs (order = trigger order) -------------
        siA = consts.tile([128, B_PER * T], FP8, tag="siA")
        siB = consts.tile([128, T], FP8, tag="siB")
        nc.sync.dma_start(siA[:, 0:512], ins["siA"][:, 0:512])
        nc.sync.dma_start(siA[:, 512:1024], ins["siA"][:, 512:1024])
        nc.sync.dma_start(siB[:, 0:1024], ins["siB"][:, 0:1024])
        # W1dr before the h1 spike halves: fc1(b0,h0)'s matmuls need it at
        # ~14us; the h1 scans it displaces have >2us of slack.
        w1dr = consts.tile([128, 16 * 256], FP8, tag="w1dr")
        nc.sync.dma_start(w1dr[:], ins["W1dr"][:])
        nc.sync.dma_start(siA[:, 1024:T], ins["siA"][:, 1024:T])
        nc.sync.dma_start(siB[:, 1024:T], ins["siB"][:, 1024:T])
        nc.sync.dma_start(siA[:, T:2 * T], ins["siA"][:, T:2 * T])
        nc.sync.dma_start(siA[:, 2 * T:3 * T], ins["siA"][:, 2 * T:3 * T])
        sip = consts.tile([128, KT * NB2], FP8, tag="sip")
        nc.sync.dma_start(sip[:], ins["sipT"][:])
        wl1 = consts.tile([128, KT * HID], FP8, tag="wl1")
        nc.sync.dma_start(wl1[:], ins["Wl1T"][:])
        nc.sync.dma_start(siA[:, 3 * T:4 * T], ins["siA"][:, 3 * T:4 * T])
        w2t = consts.tile([128, 4 * 32], FP8, tag="w2t")
        nc.sync.dma_start(w2t[:], ins["W2T"][:])
        wl2 = consts.tile([128, 4 * 32], BF16, tag="wl2")
        nc.sync.dma_start(wl2[:], ins["Wl2T"][:])

        # ---------------- PE warmup (HAM cold-start) ----------------------
        # ~32 junk matmuls keep the PE busy from ~0.5us so the HAM window
        # flips to 8/8 before real work and never re-throttles.
        pwarm = psum2.tile([128, 512], F32, tag="ps2", name="pwarm")
        for i in range(28):
            nc.tensor.matmul(pwarm[:], junk[:, 0:128], junk[:],
                             start=True, stop=True)

        # ---------------- branch-1 input psp scans (DVE, fp8 out) --------
        # combo slots: [A0 | B | A1 | B | A2 | B | A3 | B] so sample b's
        # fc1 DR rhs is combo[:, 2bT:2bT+2T] viewed as [128, 2, T].
        # A0/B run as interleaved halves (fp8 carry) so fc1(b0)'s h0 units
        # unblock ~5us earlier.
        combo = work.tile([128, 8 * T], FP8, tag="combo")
        H = 1024

        def emit_scan_half(b, hf):
            # slot 2b = A_b; scan half hf with carried initial
            c0 = 2 * b * T + hf * H
            init = 0.0 if hf == 0 else combo[:, c0 - 1:c0]
            nc.vector.tensor_tensor_scan(combo[:, c0:c0 + H], alpha_t[:, 0:H],
                                         siA[:, b * T + hf * H:b * T + (hf + 1) * H],
                                         init, AL.mult, AL.add)

        def emit_scan_B(hf):
            c0 = T + hf * H
            init = 0.0 if hf == 0 else combo[:, c0 - 1:c0]
            nc.vector.tensor_tensor_scan(combo[:, c0:c0 + H], alpha_t[:, 0:H],
                                         siB[:, hf * H:(hf + 1) * H],
                                         init, AL.mult, AL.add)

        # A0/B as interleaved halves (fc1-b0 h0 units unblock early),
        # then A1..A3; B-slot replication after B completes. A0's first
        # half runs as two 512 chunks so the chain starts on a smaller DMA.
        nc.vector.tensor_tensor_scan(combo[:, 0:512], alpha_t[:, 0:512],
                                     siA[:, 0:512], 0.0, AL.mult, AL.add)
        nc.vector.tensor_tensor_scan(combo[:, 512:1024], alpha_t[:, 0:512],
                                     siA[:, 512:1024], combo[:, 511:512],
                                     AL.mult, AL.add)
        emit_scan_B(0)
        emit_scan_half(0, 1)
        emit_scan_B(1)
        for b in range(1, B_PER):
            emit_scan_half(b, 0)
            emit_scan_half(b, 1)
        for b in range(1, B_PER):
            nc.sync.dma_start(combo[:, (2 * b + 1) * T:(2 * b + 2) * T],
                              combo[:, T:2 * T])

        w1dr4 = w1dr[:].rearrange("p (g j i) -> p g j i", g=16, j=2, i=128)
        combo3 = combo[:].rearrange("p (s t) -> p s t", t=T)
        wl1_3d = wl1[:].rearrange("p (k o) -> p k o", o=HID)
        sip_3d = sip[:].rearrange("p (k c) -> p k c", c=NB2)

        sg = [work.tile([128, 4 * T], FP8, tag=f"sg{b}", name=f"sg{b}")
              for b in range(B_PER)]
        l1 = []

        # ---------------- block emitters ---------------------------------
        # threshold units handed to DVE (late per-sample units; DVE frees up
        # once the input scans drain, ACT is otherwise the pace-setter)
        DVE_UNITS = {(2, 3, 0), (2, 3, 1), (3, 2, 1), (3, 3, 1)}

        def fc1_half(b, hf):
            """fc1 sample b, half hf: 4 m-tiles, fp8 DR, threshold."""
            if True:
                for m in range(4):
                    pu = psum1.tile([128, 1024], F32, tag="ps",
                                    name=f"fc1_{b}{m}{hf}")
                    for ch in range(2):
                        csl = slice(hf * 1024 + ch * 512,
                                    hf * 1024 + (ch + 1) * 512)
                        nc.tensor.matmul(pu[:, ch * 512:(ch + 1) * 512],
                                         w1dr4[:, b * 4 + m],
                                         combo3[:, 2 * b:2 * b + 2, csl],
                                         start=True, stop=True, perf_mode=DR)
                    dst = sg[b][:, m * T + hf * 1024:m * T + (hf + 1) * 1024]
                    if (b, m, hf) in DVE_UNITS:
                        nc.vector.tensor_scalar(dst, pu[:], THETA, None,
                                                AL.is_ge)
                    else:
                        nc.scalar.activation(dst, pu[:], AF.Sigmoid,
                                             bias=bias_sig[:], scale=SIG_SCALE)

        a1sb = [None] * 4
        a1ps = []

        def a1_block(m, copy_eng="act"):
            """branch-2 fc1 m-tile: fp8 DR over t into two 1-bank PSUM
            tiles; copied to SBUF immediately (recycling psum2) by ACT for
            the early blocks, by DVE (its post-scan idle window) for the
            late ones; the psp scan + l1 threshold run later (emit_ul1)."""
            pa = psum2.tile([128, 512], F32, tag="ps2", name=f"pa{m}a")
            pb = psum2.tile([128, 512], F32, tag="ps2", name=f"pa{m}b")
            msl = slice(m * 128, (m + 1) * 128)
            for ki in range(KT // 2):
                st, sp = (ki == 0), (ki == KT // 2 - 1)
                lhs = wl1_3d[:, 2 * ki:2 * ki + 2, msl]
                nc.tensor.matmul(pa[:], lhs,
                                 sip_3d[:, 2 * ki:2 * ki + 2, 0:512],
                                 start=st, stop=sp, perf_mode=DR)
                nc.tensor.matmul(pb[:, 0:NB2 - 512], lhs,
                                 sip_3d[:, 2 * ki:2 * ki + 2, 512:NB2],
                                 start=st, stop=sp, perf_mode=DR)
            a1ps.append((pa, pb))
            if copy_eng is not None:
                emit_a1_copy(m, copy_eng)

        def emit_a1_copy(m, eng):
            pa, pb = a1ps[m]
            ab = mid.tile([128, NB2], F32, tag="a1c", name=f"a1c{m}")
            if eng == "act":
                nc.scalar.activation(ab[:, 0:512], pa[:], AF.Copy)
                nc.scalar.activation(ab[:, 512:NB2], pb[:, 0:NB2 - 512],
                                     AF.Copy)
            else:
                nc.vector.tensor_copy(ab[:, 0:512], pa[:])
                nc.vector.tensor_copy(ab[:, 512:NB2], pb[:, 0:NB2 - 512])
            a1sb[m] = ab

        def emit_ul1(m):
            u = mid.tile([128, NB2], F32, tag="ul1", name=f"ul1{m}")
            nc.vector.tensor_tensor_scan(u[:], pat624[:], a1sb[m][:], 0.0,
                                         AL.mult, AL.add)
            lt = mid.tile([128, NB2], BF16, tag="l1", name=f"l1{m}")
            nc.vector.tensor_scalar(lt[:], u[:], THETA, None, AL.is_ge)
            l1.append(lt)

        # ---------------- main schedule -----------------------------------
        # DVE queue: A0/B halves, A1, ul1-0, A2, ul1-1, A3, ul1-2, (b2/b3
        # DVE threshold units via fc1_block), ul1-3 -- ul1 scans slot into
        # the scan chain right after their a1 block lands, which also
        # recycles the two psum2 bufs for the next a1 block.
        # per-sample h0+h1 units with a1 blocks filling the PE idle left
        # by the consumer-paced fc1 pipeline.
        fc1_half(0, 0)
        fc1_half(0, 1)
        a1_block(0)
        a1_block(1)
        fc1_half(1, 0)
        fc1_half(1, 1)
        a1_block(2)
        emit_ul1(0)
        emit_ul1(1)
        fc1_half(2, 0)
        fc1_half(2, 1)
        a1_block(3)
        emit_ul1(2)
        fc1_half(3, 0)
        fc1_half(3, 1)
        emit_ul1(3)
        # ---------------- branch-1 fc2 + psp + out, in t-quarters ---------
        vs = work.tile([128, T], F32, tag="vs")
        o1 = work.tile([128, T], BF16, tag="o1")

        def emit_quarter(q):
            qsl = slice(q * 512, (q + 1) * 512)
            puq = psum1.tile([128, 1024], F32, tag="ps", name=f"fc2q{q}")
            for m in range(4):
                for b in range(B_PER):
                    nc.tensor.matmul(puq[32 * b:32 * b + 32, 0:512],
                                     w2t[:, m * 32:(m + 1) * 32],
                                     sg[b][:, m * T + q * 512:m * T + (q + 1) * 512],
                                     start=(m == 0), stop=(m == 3),
                                     tile_position=(0, 32 * b),
                                     skip_group_check=True)
            init = 0.0 if q == 0 else vs[:, q * 512 - 1:q * 512]
            nc.vector.tensor_tensor_scan(vs[:, qsl], alpha_t[:, 0:512],
                                         puq[:, 0:512], init,
                                         AL.mult, AL.add)
            if q == 3:
                nc.vector.tensor_scalar(o1[:, qsl], vs[:, qsl], THETA, None,
                                        AL.is_ge)
            else:
                nc.scalar.activation(o1[:, qsl], vs[:, qsl], AF.Sigmoid,
                                     bias=bias_sig[:], scale=SIG_SCALE)
            nc.sync.dma_start(
                out[:, :, qsl].rearrange("b j t -> (b j) t"), o1[:, qsl])

        # ---------------- branch-2 fc2 (col-tiled) + psp + out ------------
        pl2 = psum2.tile([128, 512], F32, tag="ps2", name="pl2")
        for k in range(4):
            for b in range(B_PER):
                nc.tensor.matmul(pl2[32 * b:32 * b + 32, 0:CP],
                                 wl2[:, k * 32:(k + 1) * 32],
                                 l1[k][:, b * CP:(b + 1) * CP],
                                 start=(k == 0), stop=(k == 3),
                                 tile_position=(0, 32 * b),
                                 skip_group_check=True)
        u2 = mid.tile([128, CP], F32, tag="u2")
        nc.vector.tensor_tensor_scan(u2[:], alpha_t[:, 0:CP], pl2[:, 0:CP],
                                     0.0, AL.mult, AL.add)
        o2 = mid.tile([128, CP], BF16, tag="o2")
        nc.scalar.activation(o2[:], u2[:], AF.Sigmoid,
                             bias=bias_sig[:], scale=SIG_SCALE)
        nc.sync.dma_start(
            out[:, :, T:T + CP].rearrange("b j c -> (b j) c"), o2[:])

        for q in range(4):
            emit_quarter(q)


# ======================= host-side preparation =======================

def prep_core_inputs(si, sip, core):
    """Per-core data tensors (fp8), pre-packed into single-DMA layouts."""
    sl = si[core * B_PER:(core + 1) * B_PER]          # [4,156,2048]
    siA = np.ascontiguousarray(
        sl[:, :128, :].transpose(1, 0, 2).reshape(128, B_PER * T)
    ).astype(FP8_NP)
    siB = np.zeros((128, T), dtype=FP8_NP)
    for b in range(B_PER):
        siB[32 * b:32 * b + (C_IN - 128)] = sl[b, 128:C_IN, :]
    sp = sip[core * B_PER:(core + 1) * B_PER]         # [4,156,2048]
    sipT = np.ascontiguousarray(
        sp.transpose(2, 0, 1).reshape(KT, 128, NB2)
        .transpose(1, 0, 2).reshape(128, KT * NB2)
    ).astype(FP8_NP)
    return {"siA": siA, "siB": siB, "sipT": sipT}


def prep_shared_inputs(W1, W2, Wl1, Wl2):
    """Weight layouts, shared by all cores (all fp8 except Wl2 bf16)."""
    w1t = W1.T.astype(np.float32)                     # [156, 512]
    W1dr = np.zeros((128, 4, 4, 2, 128), dtype=np.float32)
    for b in range(B_PER):
        for m in range(4):
            W1dr[:, b, m, 0, :] = w1t[:128, m * 128:(m + 1) * 128]
            W1dr[32 * b:32 * b + (C_IN - 128), b, m, 1, :] = \
                w1t[128:C_IN, m * 128:(m + 1) * 128]
    W1dr = W1dr.reshape(128, 16 * 256).astype(FP8_NP)

    w2tt = W2.T.astype(np.float32)                    # [512, 20]
    W2T = np.zeros((128, 4 * 32), dtype=FP8_NP)
    for k in range(4):
        W2T[:, k * 32:k * 32 + OUT_DIM] = w2tt[k * 128:(k + 1) * 128]

    Wl1T = np.ascontiguousarray(
        Wl1.T.reshape(KT, 128, HID).transpose(1, 0, 2).reshape(128, KT * HID)
    ).astype(FP8_NP)

    wl2t = Wl2.T.astype(np.float32)                   # [512, 20]
    Wl2T = np.zeros((128, 4 * 32), dtype=BF16_NP)
    for k in range(4):
        Wl2T[:, k * 32:k * 32 + OUT_DIM] = wl2t[k * 128:(k + 1) * 128]
    return {"W1dr": W1dr, "W2T": W2T, "Wl1T": Wl1T, "Wl2T": Wl2T}


def make_in_maps(spike_input, W1, W2, Wl1, Wl2, perm):
    si = np.asarray(spike_input, dtype=np.float32).reshape(B, C_IN, T)
    perm = np.asarray(perm).astype(np.int64)
    sip = si[:, perm, :]                              # perm-gather (layout only)
    shared = prep_shared_inputs(np.asarray(W1, np.float32),
                                np.asarray(W2, np.float32),
                                np.asarray(Wl1, np.float32),
                                np.asarray(Wl2, np.float32))
    in_maps = []
    for core in range(N_CORES):
        m = dict(shared)
        m.update(prep_core_inputs(si, sip, core))
        in_maps.append(m)
    return in_maps


_IN_SPECS = {
    "siA": ((128, B_PER * T), FP8),
    "siB": ((128, T), FP8),
    "sipT": ((128, KT * NB2), FP8),
    "W1dr": ((128, 16 * 256), FP8),
    "W2T": ((128, 4 * 32), FP8),
    "Wl1T": ((128, KT * HID), FP8),
    "Wl2T": ((128, 4 * 32), BF16),
}


def build_bass():
    nc = bacc.Bacc("TRN2", target_bir_lowering=False, debug=False)
    ins = {}
    for name, (shape, dt) in _IN_SPECS.items():
        h = nc.dram_tensor(name, list(shape), dt, kind="ExternalInput")
        ins[name] = h[:]
    out_h = nc.dram_tensor("out", [B_PER, 32, T + CP], BF16,
                           kind="ExternalOutput")
    outs = {"out": out_h[:]}
    with tile_mod.TileContext(nc) as tc:
        build_program(tc, outs, ins)
    nc.compile()
    return nc


_NC_CACHE = None


def run(inputs, trace=False, **kw):
    """Run on the 8 NeuronCores; returns (full_output, BassKernelResults)."""
    global _NC_CACHE
    if _NC_CACHE is None:
        _NC_CACHE = build_bass()
    nc = _NC_CACHE
    in_maps = make_in_maps(**inputs)
    res = run_bass_kernel_spmd(nc, in_maps, core_ids=list(range(N_CORES)),
                               trace=trace, **kw)
    parts = [res.results[c]["out"][:, :OUT_DIM, :] for c in range(N_CORES)]
    full = np.concatenate(parts, axis=0).reshape(B, OUT_DIM, 1, 1, T + CP)
    return np.ascontiguousarray(full.astype(np.float32)), res


def kernel(**inputs):
    out, _ = run(inputs)
    return out
